# revision 23
# baseline (speedup 1.0000x reference)
"""BLinear (binarized linear) Trainium2 kernel.

Computes y = x @ sign(weight)^T / sqrt(SIZE_IN) for
x [8192, 4096] f32, weight [4096, 4096] f32 -> y [8192, 4096] f32.

Strategy: data-parallel over tokens across 8 NeuronCores. Each core gets
x^T shard [4096, 1024] and the full w^T [4096, 4096] (host does only the
layout transpose + sharding; all numerics - sign, cast, matmul, scale -
run on device). Per core:

  - x^T tiles are DMA'd, cast f32->bf16 on DVE, kept SBUF-resident.
  - w^T tiles [128, 512] are DMA'd and sign-binarized to bf16 {+-1} on ACT
    (exact in bf16, so the only quantization error is the bf16 rounding of
    x: ~1.7e-3 relative). The binarized pool is 64 tiles deep (two full
    o-chunks) so binarization runs a whole o-chunk ahead of the PE.
  - PE runs 2048 bf16 matmuls (lhsT = x^T tile [128i,128t], rhs =
    w_bin^T [128i,512o], N=512) accumulating over k into 8 PSUM banks.
    Loop nest: o-chunk outer; within a chunk, t-passes with k innermost
    (K-contiguous per bank) so the 8 accumulation groups COMPLETE
    STAGGERED - each bank's evict overlaps the next t-pass's matmuls and
    the PE never idles at chunk boundaries (idle >3.4us would also drop
    the PE clock from 2.4 to 1.2 GHz via HAM). The first o-chunk is
    k-blocked (8 k-tiles per block, t inner) because x is still streaming
    in at that point and is consumed in k order.
  - DVE evicts each finished PSUM group to SBUF with the 1/64 scale
    fused; GPSIMD DMAs the f32 result out. All fully overlapped.

Raw Bass (no TileContext - its EVSEM barrier/branch preamble does not
compile on this toolchain), explicit semaphore pipeline, fully unrolled.

NOTE on DMA semaphores: one dma_start raises its semaphore by 16
incrementally (+1 per DMA queue slice), so counts from concurrent
transfers interleave. Every DMA stream gets ONE SEM PER BUFFER SLOT and
consumers wait for exact per-slot totals.
"""

import contextlib
import sys

sys.path.insert(0, "/opt/trn_rl_repo")

import numpy as np

import concourse.bass as bass
import concourse.mybir as mybir
from concourse.bass_utils import run_bass_kernel_spmd

TOKENS = 8192
SIZE_IN = 4096
SIZE_OUT = 4096
N_CORES = 8
TC = TOKENS // N_CORES  # tokens per core

F32 = mybir.dt.float32
BF16 = mybir.dt.bfloat16


def build_nc(TC=TC, K=SIZE_IN, O=SIZE_OUT, scale=1.0 / (SIZE_IN**0.5)):
    """Build the per-core Bass program (SPMD: same program on all cores)."""
    P = 128  # partition dim / k-tile
    NT = TC // P       # t-tiles (stationary cols / psum banks): 8 full size
    NK = K // P        # k-tiles (contraction)                 : 32
    OC = 512           # o-chunk (moving free dim, one PSUM bank of f32)
    NO = O // OC       # o-chunks                              : 8
    KB = min(8, NK)    # k-block for the first o-chunk
    XB = 4             # x f32 staging depth
    WS = 8             # w f32 staging depth
    W2 = 2 * NK        # binarized w pool depth (two full o-chunks)
    YB = 4             # y staging depth
    assert NT <= 8 and NK % KB == 0

    nc = bass.Bass()
    xt = nc.declare_dram_parameter("xt", [K, TC], F32, isOutput=False)
    wt = nc.declare_dram_parameter("wt", [K, O], F32, isOutput=False)
    y = nc.declare_dram_parameter("y", [TC, O], F32, isOutput=True)

    NW = NO * NK      # total w tiles (256)
    NG = NO * NT      # total output groups (64)

    ctx = contextlib.ExitStack()
    with ctx:
        sem_warm = ctx.enter_context(nc.semaphore("sem_warm"))
        sem_xcast = ctx.enter_context(nc.semaphore("sem_xcast"))
        sem_wsign = ctx.enter_context(nc.semaphore("sem_wsign"))
        sem_wbfree = ctx.enter_context(nc.semaphore("sem_wbfree"))
        sem_grp = ctx.enter_context(nc.semaphore("sem_grp"))
        sem_evict = ctx.enter_context(nc.semaphore("sem_evict"))
        sem_xdma_s = [
            ctx.enter_context(nc.semaphore(f"sem_xdma{i}")) for i in range(XB)
        ]
        sem_wdma_s = [
            ctx.enter_context(nc.semaphore(f"sem_wdma{i}")) for i in range(WS)
        ]
        sem_ystore_s = [
            ctx.enter_context(nc.semaphore(f"sem_ystore{i}")) for i in range(YB)
        ]

        xs = [
            ctx.enter_context(nc.sbuf_tensor(f"xs{i}", [P, TC], F32))
            for i in range(XB)
        ]
        xb = [
            ctx.enter_context(nc.sbuf_tensor(f"xb{k}", [P, TC], BF16))
            for k in range(NK)
        ]
        ws = [
            ctx.enter_context(nc.sbuf_tensor(f"ws{i}", [P, OC], F32))
            for i in range(WS)
        ]
        wb = [
            ctx.enter_context(nc.sbuf_tensor(f"wb{i}", [P, OC], BF16))
            for i in range(W2)
        ]
        ys = [
            ctx.enter_context(nc.sbuf_tensor(f"ys{i}", [P, OC], F32))
            for i in range(YB)
        ]
        zb = ctx.enter_context(nc.sbuf_tensor("zb", [P, OC], BF16))
        ps = [
            ctx.enter_context(nc.psum_tensor(f"ps{t}", [P, OC], F32))
            for t in range(NT)
        ]

        # tile j's wb-slot release count on sem_wbfree: tiles with
        # k == NK-1 signal completion via sem_grp instead (a matmul can
        # carry only ONE sem update, and those carry the group inc).
        def wbfree_count(jj):
            return (jj + 1) - jj // NK

        with nc.Block() as block:

            @block.sync
            def _(sp: bass.BassEngine):
                def w_load(j):
                    oc, kk = divmod(j, NK)
                    if j >= WS:
                        sp.wait_ge(sem_wsign, j - WS + 1)
                    sp.dma_start(
                        out=ws[j % WS][:],
                        in_=wt[kk * P : (kk + 1) * P, oc * OC : (oc + 1) * OC],
                    ).then_inc(sem_wdma_s[j % WS], 16)

                # Interleave x loads with the first NK w loads so oc=0's
                # k-blocks get (x, w) tile pairs in lockstep (w first:
                # the first matmul's critical path is w0 -> sign -> MM).
                for k in range(NK):
                    if k < NW:
                        w_load(k)
                    if k >= XB:
                        sp.wait_ge(sem_xcast, k - XB + 1)
                    sp.dma_start(
                        out=xs[k % XB][:],
                        in_=xt[k * P : (k + 1) * P, :],
                    ).then_inc(sem_xdma_s[k % XB], 16)
                for j in range(NK, NW):
                    w_load(j)

            @block.scalar
            def _(act: bass.BassEngine):
                # Signs, with y-store DMAs (HWDGE) interleaved: store g is
                # issued near sign j = W2 + 4g, well after evict g fires
                # and well before the ys slot is needed again. Stores live
                # here (not GPSIMD/SWDGE) because the kernel-exit SWDGE
                # drain costs ~6us.
                def y_store(g):
                    oc, t = divmod(g, NT)
                    act.wait_ge(sem_evict, g + 1)
                    act.dma_start(
                        out=y[t * P : (t + 1) * P, oc * OC : (oc + 1) * OC],
                        in_=ys[g % YB][:],
                    ).then_inc(sem_ystore_s[g % YB], 16)

                n_stored = 0
                for j in range(NW):
                    act.wait_ge(sem_wdma_s[j % WS], 16 * (j // WS + 1))
                    if j >= W2:
                        jj = j - W2
                        if jj % NK == NK - 1:
                            act.wait_ge(sem_grp, (jj // NK + 1) * NT)
                        else:
                            act.wait_ge(sem_wbfree, wbfree_count(jj))
                        if (j - W2) % 4 == 0 and n_stored < NG:
                            y_store(n_stored)
                            n_stored += 1
                    act.sign(wb[j % W2][:], ws[j % WS][:]).then_inc(sem_wsign)
                for g in range(n_stored, NG):
                    y_store(g)
                for i in range(YB):
                    uses = (NG - 1 - i) // YB + 1
                    act.wait_ge(sem_ystore_s[i], 16 * uses)

            @block.vector
            def _(dve: bass.BassEngine):
                # zero the PE warmup operand first - costs nothing (the
                # first cast waits on the first x DMA anyway)
                dve.memset(zb[:], 0.0).then_inc(sem_warm)
                for k in range(NK):
                    dve.wait_ge(sem_xdma_s[k % XB], 16 * (k // XB + 1))
                    dve.tensor_copy(xb[k][:], xs[k % XB][:]).then_inc(sem_xcast)
                for g in range(NG):
                    dve.wait_ge(sem_grp, g + 1)
                    if g >= YB:
                        dve.wait_ge(sem_ystore_s[g % YB], 16 * (g // YB))
                    dve.tensor_scalar_mul(
                        ys[g % YB][:], ps[g % NT][:], scale
                    ).then_inc(sem_evict)

            @block.tensor
            def _(pe: bass.BassEngine):
                # Warmup: dummy matmuls on zeros while the first x/w tiles
                # stream in. Keeps the PE's HAM activity window busy so the
                # real stream runs at 2.4 GHz from its first matmul (cold
                # PE is clocked 1.2 GHz; re-warming takes ~3.4us of work).
                WU = 28
                pe.wait_ge(sem_warm, 1)
                for _ in range(WU):
                    pe.matmul(
                        ps[0][:], zb[:, :P], zb[:], start=True, stop=True
                    )

                def mm(oc, t, k):
                    j = oc * NK + k
                    if t == 0:
                        pe.wait_ge(sem_wsign, j + 1)
                        if oc == 0:
                            pe.wait_ge(sem_xcast, k + 1)
                    if k == 0 and oc >= 1:
                        # bank t's previous tenant (oc-1, t) must be evicted
                        pe.wait_ge(sem_evict, (oc - 1) * NT + t + 1)
                    ins = pe.matmul(
                        ps[t][:],
                        xb[k][:, t * P : (t + 1) * P],
                        wb[j % W2][:],
                        start=(k == 0),
                        stop=(k == NK - 1),
                    )
                    if k == NK - 1:
                        ins.then_inc(sem_grp)  # group (oc, t) complete
                    elif t == NT - 1:
                        ins.then_inc(sem_wbfree)  # tile j's last use

                # oc = 0: x is still streaming in; consume it in k order
                # via k-blocks (t inner within a block).
                for kb in range(NK // KB):
                    for t in range(NT):
                        for k in range(kb * KB, (kb + 1) * KB):
                            mm(0, t, k)
                # oc >= 1: t-passes, k innermost -> groups complete
                # staggered, evicts/stores fully overlap the matmul stream.
                for oc in range(1, NO):
                    for t in range(NT):
                        for k in range(NK):
                            mm(oc, t, k)

    return nc


_NC_CACHE = {}


def _get_nc(key):
    if key not in _NC_CACHE:
        _NC_CACHE[key] = build_nc(*key)
    return _NC_CACHE[key]


def _make_in_maps(x, weight):
    xt_full = np.ascontiguousarray(x.T.astype(np.float32))      # [K, TOKENS]
    wt = np.ascontiguousarray(weight.T.astype(np.float32))      # [K, O]
    return [
        {
            "xt": np.ascontiguousarray(xt_full[:, c * TC : (c + 1) * TC]),
            "wt": wt,
        }
        for c in range(N_CORES)
    ]


def kernel(x: np.ndarray, weight: np.ndarray) -> np.ndarray:
    assert x.shape == (TOKENS, SIZE_IN) and weight.shape == (SIZE_OUT, SIZE_IN)
    nc = _get_nc((TC, SIZE_IN, SIZE_OUT, 1.0 / (SIZE_IN**0.5)))
    in_maps = _make_in_maps(x, weight)
    res = run_bass_kernel_spmd(nc, in_maps, list(range(N_CORES)))
    out = np.concatenate([res.results[c]["y"] for c in range(N_CORES)], axis=0)
    return out.astype(np.float32)


def _install_ntff_hook():
    """Register the axon NTFF profile hook (the image's antenv package
    lacks axon_hooks, so boot degraded silently; re-create it here)."""
    import types

    if "antenv.axon_hooks" not in sys.modules:
        mod = types.ModuleType("antenv.axon_hooks")
        holder = {"fn": None}
        mod.set_axon_ntff_profile_hook = lambda h: holder.__setitem__("fn", h)
        mod.get_axon_ntff_profile_hook = lambda: holder["fn"]
        sys.modules["antenv.axon_hooks"] = mod
    import antenv

    sys.modules["antenv"].axon_hooks = sys.modules["antenv.axon_hooks"]
    if sys.modules["antenv.axon_hooks"].get_axon_ntff_profile_hook() is None:
        if "/root/.axon_site" not in sys.path:
            sys.path.insert(0, "/root/.axon_site")
        from trn_agent_boot.trn_boot import _ntff_profile_via_ctypes

        sys.modules["antenv.axon_hooks"].set_axon_ntff_profile_hook(
            _ntff_profile_via_ctypes("/opt/axon/libaxon_pjrt.so")
        )
    # zero-egress container: stub the artifact upload the trace path does
    import concourse.bass_utils as bu

    bu.upload_artifacts = lambda tmpdir: f"local://{tmpdir}"


def profile(np_inputs, trace_cores=(0,), tmpdir=None):
    """Timed run with NTFF profiling; returns exec_time_ns (or None)."""
    nc = _get_nc((TC, SIZE_IN, SIZE_OUT, 1.0 / (SIZE_IN**0.5)))
    in_maps = _make_in_maps(np_inputs["x"], np_inputs["weight"])
    try:
        _install_ntff_hook()
        res = run_bass_kernel_spmd(
            nc,
            in_maps,
            list(range(N_CORES)),
            trace=True,
            trace_cores=list(trace_cores),
            tmpdir=tmpdir,
        )
        return res.exec_time_ns
    except Exception as e:  # noqa: BLE001
        print(f"profile failed: {e!r}")
        return None


# revision 25
# speedup vs baseline: 1.0469x; 1.0469x over previous
"""BLinear (binarized linear) Trainium2 kernel.

Computes y = x @ sign(weight)^T / sqrt(SIZE_IN) for
x [8192, 4096] f32, weight [4096, 4096] f32 -> y [8192, 4096] f32.

Strategy: data-parallel over tokens across 8 NeuronCores. Each core gets
x^T shard [4096, 1024] and the full w^T [4096, 4096] (host does only the
layout transpose + sharding; all numerics - sign, cast, matmul, scale -
run on device). Per core:

  - x^T tiles are DMA'd, cast f32->bf16 on DVE, kept SBUF-resident.
  - w^T tiles [128, 512] are DMA'd and sign-binarized to bf16 {+-1} on ACT
    (exact in bf16, so the only quantization error is the bf16 rounding of
    x: ~1.7e-3 relative). The binarized pool is 64 tiles deep (two full
    o-chunks) so binarization runs a whole o-chunk ahead of the PE.
  - PE runs 2048 bf16 matmuls (lhsT = x^T tile [128i,128t], rhs =
    w_bin^T [128i,512o], N=512) accumulating over k into 8 PSUM banks.
    Loop nest: o-chunk outer; within a chunk, t-passes with k innermost
    (K-contiguous per bank) so the 8 accumulation groups COMPLETE
    STAGGERED - each bank's evict overlaps the next t-pass's matmuls and
    the PE never idles at chunk boundaries (idle >3.4us would also drop
    the PE clock from 2.4 to 1.2 GHz via HAM). The first o-chunk is
    k-blocked (8 k-tiles per block, t inner) because x is still streaming
    in at that point and is consumed in k order.
  - DVE evicts each finished PSUM group to SBUF with the 1/64 scale
    fused; GPSIMD DMAs the f32 result out. All fully overlapped.

Raw Bass (no TileContext - its EVSEM barrier/branch preamble does not
compile on this toolchain), explicit semaphore pipeline, fully unrolled.

NOTE on DMA semaphores: one dma_start raises its semaphore by 16
incrementally (+1 per DMA queue slice), so counts from concurrent
transfers interleave. Every DMA stream gets ONE SEM PER BUFFER SLOT and
consumers wait for exact per-slot totals.
"""

import contextlib
import sys

sys.path.insert(0, "/opt/trn_rl_repo")

import numpy as np

import concourse.bass as bass
import concourse.mybir as mybir
from concourse.bass_utils import run_bass_kernel_spmd

TOKENS = 8192
SIZE_IN = 4096
SIZE_OUT = 4096
N_CORES = 8
TC = TOKENS // N_CORES  # tokens per core

F32 = mybir.dt.float32
BF16 = mybir.dt.bfloat16


def build_nc(TC=TC, K=SIZE_IN, O=SIZE_OUT, scale=1.0 / (SIZE_IN**0.5)):
    """Build the per-core Bass program (SPMD: same program on all cores)."""
    P = 128  # partition dim / k-tile
    NT = TC // P       # t-tiles (stationary cols / psum banks): 8 full size
    NK = K // P        # k-tiles (contraction)                 : 32
    OC = 512           # o-chunk (moving free dim, one PSUM bank of f32)
    NO = O // OC       # o-chunks                              : 8
    KB = min(4, NK)    # k-block for the first o-chunk (small: keeps the
    #                    oc=0 x-bandwidth-deficit stalls under the ~3.4us
    #                    HAM re-throttle window)
    XB = 4             # x f32 staging depth
    WS = 8             # w f32 staging depth
    W2 = 2 * NK        # binarized w pool depth (two full o-chunks)
    YB = 12            # y staging depth (deep: evicts must never wait on
    #                    the bursty store pattern or the PE stalls)
    assert NT <= 8 and NK % KB == 0

    nc = bass.Bass()
    xt = nc.declare_dram_parameter("xt", [K, TC], F32, isOutput=False)
    wt = nc.declare_dram_parameter("wt", [K, O], F32, isOutput=False)
    y = nc.declare_dram_parameter("y", [TC, O], F32, isOutput=True)

    NW = NO * NK      # total w tiles (256)
    NG = NO * NT      # total output groups (64)

    ctx = contextlib.ExitStack()
    with ctx:
        sem_warm = ctx.enter_context(nc.semaphore("sem_warm"))
        sem_xcast = ctx.enter_context(nc.semaphore("sem_xcast"))
        sem_wsign = ctx.enter_context(nc.semaphore("sem_wsign"))
        sem_wbfree = ctx.enter_context(nc.semaphore("sem_wbfree"))
        sem_grp = ctx.enter_context(nc.semaphore("sem_grp"))
        sem_evict = ctx.enter_context(nc.semaphore("sem_evict"))
        sem_xdma_s = [
            ctx.enter_context(nc.semaphore(f"sem_xdma{i}")) for i in range(XB)
        ]
        sem_wdma_s = [
            ctx.enter_context(nc.semaphore(f"sem_wdma{i}")) for i in range(WS)
        ]
        sem_ystore_s = [
            ctx.enter_context(nc.semaphore(f"sem_ystore{i}")) for i in range(YB)
        ]

        xs = [
            ctx.enter_context(nc.sbuf_tensor(f"xs{i}", [P, TC], F32))
            for i in range(XB)
        ]
        xb = [
            ctx.enter_context(nc.sbuf_tensor(f"xb{k}", [P, TC], BF16))
            for k in range(NK)
        ]
        ws = [
            ctx.enter_context(nc.sbuf_tensor(f"ws{i}", [P, OC], F32))
            for i in range(WS)
        ]
        wb = [
            ctx.enter_context(nc.sbuf_tensor(f"wb{i}", [P, OC], BF16))
            for i in range(W2)
        ]
        ys = [
            ctx.enter_context(nc.sbuf_tensor(f"ys{i}", [P, OC], F32))
            for i in range(YB)
        ]
        zb = ctx.enter_context(nc.sbuf_tensor("zb", [P, OC], BF16))
        ps = [
            ctx.enter_context(nc.psum_tensor(f"ps{t}", [P, OC], F32))
            for t in range(NT)
        ]

        # tile j's wb-slot release count on sem_wbfree: tiles with
        # k == NK-1 signal completion via sem_grp instead (a matmul can
        # carry only ONE sem update, and those carry the group inc).
        def wbfree_count(jj):
            return (jj + 1) - jj // NK

        with nc.Block() as block:

            @block.sync
            def _(sp: bass.BassEngine):
                def w_load(j):
                    oc, kk = divmod(j, NK)
                    if j >= WS:
                        sp.wait_ge(sem_wsign, j - WS + 1)
                    sp.dma_start(
                        out=ws[j % WS][:],
                        in_=wt[kk * P : (kk + 1) * P, oc * OC : (oc + 1) * OC],
                    ).then_inc(sem_wdma_s[j % WS], 16)

                # Interleave x loads with the first NK w loads so oc=0's
                # k-blocks get (x, w) tile pairs in lockstep (w first:
                # the first matmul's critical path is w0 -> sign -> MM).
                for k in range(NK):
                    if k < NW:
                        w_load(k)
                    if k >= XB:
                        sp.wait_ge(sem_xcast, k - XB + 1)
                    sp.dma_start(
                        out=xs[k % XB][:],
                        in_=xt[k * P : (k + 1) * P, :],
                    ).then_inc(sem_xdma_s[k % XB], 16)
                for j in range(NK, NW):
                    w_load(j)

            @block.scalar
            def _(act: bass.BassEngine):
                # Signs, with y-store DMAs (HWDGE) interleaved: store g is
                # issued near sign j = W2 + 4g, well after evict g fires
                # and well before the ys slot is needed again. Stores live
                # here (not GPSIMD/SWDGE) because the kernel-exit SWDGE
                # drain costs ~6us.
                def y_store(g):
                    oc, t = divmod(g, NT)
                    act.wait_ge(sem_evict, g + 1)
                    act.dma_start(
                        out=y[t * P : (t + 1) * P, oc * OC : (oc + 1) * OC],
                        in_=ys[g % YB][:],
                    ).then_inc(sem_ystore_s[g % YB], 16)

                n_stored = 0
                for j in range(NW):
                    act.wait_ge(sem_wdma_s[j % WS], 16 * (j // WS + 1))
                    if j >= W2:
                        jj = j - W2
                        if jj % NK == NK - 1:
                            act.wait_ge(sem_grp, (jj // NK + 1) * NT)
                        else:
                            act.wait_ge(sem_wbfree, wbfree_count(jj))
                        if (j - W2) % 4 == 0 and n_stored < NG:
                            y_store(n_stored)
                            n_stored += 1
                    act.sign(wb[j % W2][:], ws[j % WS][:]).then_inc(sem_wsign)
                for g in range(n_stored, NG):
                    y_store(g)
                for i in range(min(YB, NG)):
                    uses = (NG - 1 - i) // YB + 1
                    act.wait_ge(sem_ystore_s[i], 16 * uses)

            @block.vector
            def _(dve: bass.BassEngine):
                # zero the PE warmup operand first - costs nothing (the
                # first cast waits on the first x DMA anyway)
                dve.memset(zb[:], 0.0).then_inc(sem_warm)
                for k in range(NK):
                    dve.wait_ge(sem_xdma_s[k % XB], 16 * (k // XB + 1))
                    dve.tensor_copy(xb[k][:], xs[k % XB][:]).then_inc(sem_xcast)
                for g in range(NG):
                    dve.wait_ge(sem_grp, g + 1)
                    if g >= YB:
                        dve.wait_ge(sem_ystore_s[g % YB], 16 * (g // YB))
                    dve.tensor_scalar_mul(
                        ys[g % YB][:], ps[g % NT][:], scale
                    ).then_inc(sem_evict)

            @block.tensor
            def _(pe: bass.BassEngine):
                # Warmup: dummy matmuls on zeros while the first x/w tiles
                # stream in. Keeps the PE's HAM activity window busy so the
                # real stream runs at 2.4 GHz from its first matmul (cold
                # PE is clocked 1.2 GHz; re-warming takes ~3.4us of work).
                WU = 28
                pe.wait_ge(sem_warm, 1)
                for _ in range(WU):
                    pe.matmul(
                        ps[0][:], zb[:, :P], zb[:], start=True, stop=True
                    )

                def mm(oc, t, k):
                    j = oc * NK + k
                    if t == 0:
                        pe.wait_ge(sem_wsign, j + 1)
                        if oc == 0:
                            pe.wait_ge(sem_xcast, k + 1)
                    if k == 0 and oc >= 1:
                        # bank t's previous tenant (oc-1, t) must be evicted
                        pe.wait_ge(sem_evict, (oc - 1) * NT + t + 1)
                    ins = pe.matmul(
                        ps[t][:],
                        xb[k][:, t * P : (t + 1) * P],
                        wb[j % W2][:],
                        start=(k == 0),
                        stop=(k == NK - 1),
                    )
                    if k == NK - 1:
                        ins.then_inc(sem_grp)  # group (oc, t) complete
                    elif t == NT - 1:
                        ins.then_inc(sem_wbfree)  # tile j's last use

                # oc = 0: x is still streaming in; consume it in k order
                # via k-blocks (t inner within a block).
                for kb in range(NK // KB):
                    for t in range(NT):
                        for k in range(kb * KB, (kb + 1) * KB):
                            mm(0, t, k)
                # oc >= 1: t-passes, k innermost -> groups complete
                # staggered, evicts/stores fully overlap the matmul stream.
                for oc in range(1, NO):
                    for t in range(NT):
                        for k in range(NK):
                            mm(oc, t, k)

    return nc


_NC_CACHE = {}


def _get_nc(key):
    if key not in _NC_CACHE:
        _NC_CACHE[key] = build_nc(*key)
    return _NC_CACHE[key]


def _make_in_maps(x, weight):
    xt_full = np.ascontiguousarray(x.T.astype(np.float32))      # [K, TOKENS]
    wt = np.ascontiguousarray(weight.T.astype(np.float32))      # [K, O]
    return [
        {
            "xt": np.ascontiguousarray(xt_full[:, c * TC : (c + 1) * TC]),
            "wt": wt,
        }
        for c in range(N_CORES)
    ]


def kernel(x: np.ndarray, weight: np.ndarray) -> np.ndarray:
    assert x.shape == (TOKENS, SIZE_IN) and weight.shape == (SIZE_OUT, SIZE_IN)
    nc = _get_nc((TC, SIZE_IN, SIZE_OUT, 1.0 / (SIZE_IN**0.5)))
    in_maps = _make_in_maps(x, weight)
    res = run_bass_kernel_spmd(nc, in_maps, list(range(N_CORES)))
    out = np.concatenate([res.results[c]["y"] for c in range(N_CORES)], axis=0)
    return out.astype(np.float32)


def _install_ntff_hook():
    """Register the axon NTFF profile hook (the image's antenv package
    lacks axon_hooks, so boot degraded silently; re-create it here)."""
    import types

    if "antenv.axon_hooks" not in sys.modules:
        mod = types.ModuleType("antenv.axon_hooks")
        holder = {"fn": None}
        mod.set_axon_ntff_profile_hook = lambda h: holder.__setitem__("fn", h)
        mod.get_axon_ntff_profile_hook = lambda: holder["fn"]
        sys.modules["antenv.axon_hooks"] = mod
    import antenv

    sys.modules["antenv"].axon_hooks = sys.modules["antenv.axon_hooks"]
    if sys.modules["antenv.axon_hooks"].get_axon_ntff_profile_hook() is None:
        if "/root/.axon_site" not in sys.path:
            sys.path.insert(0, "/root/.axon_site")
        from trn_agent_boot.trn_boot import _ntff_profile_via_ctypes

        sys.modules["antenv.axon_hooks"].set_axon_ntff_profile_hook(
            _ntff_profile_via_ctypes("/opt/axon/libaxon_pjrt.so")
        )
    # zero-egress container: stub the artifact upload the trace path does
    import concourse.bass_utils as bu

    bu.upload_artifacts = lambda tmpdir: f"local://{tmpdir}"


def profile(np_inputs, trace_cores=(0,), tmpdir=None):
    """Timed run with NTFF profiling; returns exec_time_ns (or None)."""
    nc = _get_nc((TC, SIZE_IN, SIZE_OUT, 1.0 / (SIZE_IN**0.5)))
    in_maps = _make_in_maps(np_inputs["x"], np_inputs["weight"])
    try:
        _install_ntff_hook()
        res = run_bass_kernel_spmd(
            nc,
            in_maps,
            list(range(N_CORES)),
            trace=True,
            trace_cores=list(trace_cores),
            tmpdir=tmpdir,
        )
        return res.exec_time_ns
    except Exception as e:  # noqa: BLE001
        print(f"profile failed: {e!r}")
        return None
